# revision 38
# baseline (speedup 1.0000x reference)
"""DCRNN (nn_DCRNNModel) forward pass on 8 Trainium2 NeuronCores.

Data-parallel over batch (B=32 -> 4 chains/core), no collectives. All
weights plus the four diffusion operators S0, S0^2, S1, S1^2 (S^2
precomputed on host) stay SBUF-resident in fp16; every PE matmul runs in
fp16 (1 cycle/row at any moving size), PSUM accumulates fp32.

Structure per step (enc0+enc1 / dec0+dec1):
- Layer 1 diffuses its node-major pack zn1=[h1|h0'] per chain; the V tiles
  carry S^m h0' in rows 64:128, which layer 0 of the NEXT step reuses as
  its whole h-part diffusion (row-shifted zero-padded weights), so layer 0
  performs no hidden-state diffusion of its own.
- Encoder x-terms (tiny 2-feature inputs) are folded on the HOST into
  per-chain/per-pair projection-contribution tiles (xca/xcb), added into
  the PSUM groups via one identity matmul each; decoder y-terms diffuse a
  4-feature y-quad pack on device (4 sets/step).
- Gate B diffuses only r*h, pair-packed two chains per 128-wide pack, with
  block-diagonal pair-merged projection weights; its x-terms reuse the V
  tiles through zero-padded K=128 weights so every matmul operand sits at
  partition base 0 (HW: lhsT/rhs must share base partition; PE tile row
  must stay fixed inside an accumulation group).
- Sigmoid/tanh are split into 64-partition halves so r, u, c land at
  base 0 (TensorTensor requires equal base partitions for SBUF inputs);
  GRU runs per-chain in fp16 on DVE; h' returns to node-major via fp16
  is_transpose; PSUM->SBUF copies alternate Scalar/Vector engines.
- Emission is stage-interleaved (gA both pairs, gB-diff/proj sequenced,
  next-step x DMAs and xc loads placed at the two GRU-chain stall points)
  so the PE stays >97% occupied; loops are ordered so consecutive matmuls
  share the stationary operand (one weight load per kt).
"""
import sys
import os
import time as _time
import numpy as np

sys.path.insert(0, '/opt/trn_rl_repo')

N = 1024
B = 32
T_FULL = 12
HOR_FULL = 12
HID = 64
N_CORES = 8
CHAINS = 4
KT = 8
P = 128

_cache = {}

L1LAYERS = (('enc1', 64), ('dec1', 64))


def _build(T, HOR):
    import concourse.bacc as bacc
    import concourse.tile as tile
    from concourse import mybir

    F32 = mybir.dt.float32
    F16 = mybir.dt.float16
    AF = mybir.ActivationFunctionType

    _t0 = _time.time()
    nc = bacc.Bacc('TRN2', target_bir_lowering=False, debug=False,
                   num_devices=N_CORES)

    # ---- DRAM declarations ----
    d = {}
    for m in range(4):
        d[f's_t{m}'] = nc.dram_tensor(f's_t{m}', [P, KT, N], F16,
                                      kind='ExternalInput').ap()
    d['ident'] = nc.dram_tensor('ident', [P, P], F16, kind='ExternalInput').ap()
    d['xT'] = nc.dram_tensor('xT', [CHAINS, T, 2, N], F16,
                             kind='ExternalInput').ap()
    d['xca'] = nc.dram_tensor('xca', [T, CHAINS, P, N], F16,
                              kind='ExternalInput').ap()
    d['xcb'] = nc.dram_tensor('xcb', [T, 2, P, N], F16,
                              kind='ExternalInput').ap()
    d['zeros'] = nc.dram_tensor('zeros', [P, N], F16, kind='ExternalInput').ap()
    # layer-0 weights (enc0 I=2, dec0 I=1)
    for L, I in (('enc0', 2), ('dec0', 1)):
        d[L + '_wa_id'] = nc.dram_tensor(L + '_wa_id', [P, P], F16,
                                         kind='ExternalInput').ap()
        d[L + '_wa_hs'] = nc.dram_tensor(L + '_wa_hs', [P, 4, P], F16,
                                         kind='ExternalInput').ap()
        if L == 'dec0':
            d[L + '_waq'] = nc.dram_tensor(L + '_waq', [4 * I, 4, CHAINS, P],
                                           F16, kind='ExternalInput').ap()
            d[L + '_wcq'] = nc.dram_tensor(L + '_wcq', [4 * I, 4, 2, P], F16,
                                           kind='ExternalInput').ap()
        d[L + '_wc0h'] = nc.dram_tensor(L + '_wc0h', [P, P], F16,
                                        kind='ExternalInput').ap()
        d[L + '_wcr'] = nc.dram_tensor(L + '_wcr', [P, 4, P], F16,
                                       kind='ExternalInput').ap()
        d[L + '_bru'] = nc.dram_tensor(L + '_bru', [HID, 2], F32,
                                       kind='ExternalInput').ap()
        d[L + '_bc'] = nc.dram_tensor(L + '_bc', [HID, 1], F32,
                                      kind='ExternalInput').ap()
    d['enc0_wc0x'] = nc.dram_tensor('enc0_wc0x', [P, 2, P], F16,
                                    kind='ExternalInput').ap()
    d['dec0_wa0y'] = nc.dram_tensor('dec0_wa0y', [1, P], F16,
                                    kind='ExternalInput').ap()
    d['dec0_wc0y'] = nc.dram_tensor('dec0_wc0y', [1, 2, P], F16,
                                    kind='ExternalInput').ap()
    # layer-1 weights (I=64)
    for L, I in L1LAYERS:
        d[L + '_wa'] = nc.dram_tensor(L + '_wa', [P, 5, P], F16,
                                      kind='ExternalInput').ap()
        d[L + '_wa0x'] = nc.dram_tensor(L + '_wa0x', [HID, P], F16,
                                        kind='ExternalInput').ap()
        d[L + '_wc0h'] = nc.dram_tensor(L + '_wc0h', [P, P], F16,
                                        kind='ExternalInput').ap()
        d[L + '_wc0x'] = nc.dram_tensor(L + '_wc0x', [HID, 2, P], F16,
                                        kind='ExternalInput').ap()
        d[L + '_wcx'] = nc.dram_tensor(L + '_wcx', [P, 4, 2, P], F16,
                                       kind='ExternalInput').ap()
        d[L + '_wcr'] = nc.dram_tensor(L + '_wcr', [P, 4, P], F16,
                                       kind='ExternalInput').ap()
        d[L + '_bru'] = nc.dram_tensor(L + '_bru', [HID, 2], F32,
                                       kind='ExternalInput').ap()
        d[L + '_bc'] = nc.dram_tensor(L + '_bc', [HID, 1], F32,
                                      kind='ExternalInput').ap()
    d['fcnw1'] = nc.dram_tensor('fcnw1', [HID, 1], F16, kind='ExternalInput').ap()
    d['fcnb1'] = nc.dram_tensor('fcnb1', [1, 1], F32, kind='ExternalInput').ap()
    d['fcnb128'] = nc.dram_tensor('fcnb128', [P, 1], F32, kind='ExternalInput').ap()
    d_out = nc.dram_tensor('out', [HOR, CHAINS, N], F16, kind='ExternalOutput').ap()

    with tile.TileContext(nc) as tc:
        with tc.tile_pool(name='const', bufs=1) as const, \
             tc.tile_pool(name='state', bufs=1) as state, \
             tc.tile_pool(name='vl', bufs=16) as vlp, \
             tc.tile_pool(name='vp', bufs=4) as vpp, \
             tc.tile_pool(name='xc', bufs=6) as xcp, \
             tc.tile_pool(name='rr', bufs=2) as rrp, \
             tc.tile_pool(name='uu', bufs=4) as uup, \
             tc.tile_pool(name='rh', bufs=2) as rhp, \
             tc.tile_pool(name='ct', bufs=4) as ctp, \
             tc.tile_pool(name='pd', bufs=2, space='PSUM') as pdp, \
             tc.tile_pool(name='prj', bufs=2, space='PSUM') as prj:

            # ---- load constants ----
            cst = {}
            for name, dd in d.items():
                if name in ('xT', 'xca', 'xcb', 'zeros'):
                    continue
                t = const.tile(list(dd.shape), dd.dtype, tag=name,
                               name='cst_' + name)
                nc.sync.dma_start(t[:], dd[:])
                cst[name] = t
            s_t = [cst[f's_t{m}'] for m in range(4)]
            ident = cst['ident']
            d_zeros = d['zeros']
            zn_zeros = d_zeros.rearrange('p (kt f) -> p kt f', kt=KT)

            # ---- persistent state ----
            zT0, zT1, zn1 = [], [], []
            for c in range(CHAINS):
                zT0.append(state.tile([P, N], F16, tag=f'zT0_{c}', name=f'zT0_{c}'))
                zT1.append(state.tile([HID, N], F16, tag=f'zT1_{c}', name=f'zT1_{c}'))
                zn1.append(state.tile([P, KT, P], F16, tag=f'zn1_{c}', name=f'zn1_{c}'))
                nc.sync.dma_start(zT0[c][:], d_zeros[:])
                nc.sync.dma_start(zT1[c][:], d_zeros[0:HID, :])
                nc.sync.dma_start(zn1[c][:], zn_zeros[:])
            ytmp = [state.tile([1, N], F16, tag=f'ytmp_{c}', name=f'ytmp_{c}')
                    for c in range(CHAINS)]
            zr = [state.tile([P, KT, P], F16, tag=f'zr_{pr}', name=f'zr_{pr}')
                  for pr in range(2)]
            yq = state.tile([P, KT, CHAINS], F16, tag='yq', name='yq')

            vl1 = {}     # (c, m) -> V tile of layer-1 diffusion (prev step)

            copy_rr = [0]

            def copy_psum(dst, src):
                copy_rr[0] += 1
                if copy_rr[0] % 2:
                    nc.scalar.activation(dst, src, AF.Copy)
                else:
                    nc.vector.tensor_copy(dst, src)

            def diffuse(lhs_fn, m, pool, tag, nfeat=P):
                """V = (S_m z)^T feature-major [nfeat, N] fp16 SBUF tile.
                lhs_fn(kt) -> lhsT AP [128, nfeat]."""
                pd = pdp.tile([P, N], F32, tag='pd')
                stt = s_t[m]
                for kt in range(KT):
                    for ms in range(2):
                        sl = slice(ms * 512, (ms + 1) * 512)
                        nc.tensor.matmul(pd[0:nfeat, sl], lhs_fn(kt),
                                         stt[:, kt, sl],
                                         start=(kt == 0), stop=(kt == KT - 1))
                v = pool.tile([nfeat, N], F16, tag=tag)
                copy_psum(v[:], pd[0:nfeat, :])
                return v

            def proj_cont(ptile, psl, mms):
                """Continue accumulating into an already-closed PSUM group."""
                nmm = len(mms)
                for j, (lhs, rhs) in enumerate(mms):
                    for ms in range(2):
                        sl = slice(ms * 512, (ms + 1) * 512)
                        nc.tensor.matmul(ptile[psl, sl], lhs, rhs[:, sl],
                                         start=False, stop=(j == nmm - 1),
                                         skip_group_check=True)

            def proj(ptile, psl, mms):
                """Accumulate terms into ptile[psl, :]; term-outer order so
                both 512-slices of one term share the weight load."""
                nmm = len(mms)
                for j, (lhs, rhs) in enumerate(mms):
                    for ms in range(2):
                        sl = slice(ms * 512, (ms + 1) * 512)
                        nc.tensor.matmul(ptile[psl, sl], lhs, rhs[:, sl],
                                         start=(j == 0), stop=(j == nmm - 1))

            def sigmoid_ru(L, pa):
                rr = rrp.tile([HID, N], F16, tag='rr')
                uu = uup.tile([HID, N], F16, tag='uu')
                nc.scalar.activation(rr[:], pa[0:HID, :], AF.Sigmoid,
                                     bias=cst[L + '_bru'][:, 0:1])
                nc.scalar.activation(uu[:], pa[HID:P, :], AF.Sigmoid,
                                     bias=cst[L + '_bru'][:, 1:2])
                return rr, uu

            def gru_and_transpose(zh, ct_c, uu, dst_znc, dst_off):
                """h' = c + u*(h-c); transpose h' into dst_znc[:, :, off]."""
                t1 = rrp.tile([HID, N], F16, tag='tg')
                nc.vector.tensor_sub(t1[:], zh[0:HID, :], ct_c[:])
                nc.vector.tensor_mul(t1[:], t1[:], uu[:])
                nc.vector.tensor_add(zh[0:HID, :], ct_c[:], t1[:])
                pth = pdp.tile([P, KT, HID], F16, tag='pd')
                for kt in range(KT):
                    nc.tensor.transpose(pth[:, kt, :],
                                        zh[0:HID, kt * P:(kt + 1) * P],
                                        ident[0:HID, 0:HID])
                copy_psum(dst_znc[:, :, dst_off:dst_off + HID], pth[:])

            def gA_l0(L, c0, vq, skip_h, pa_pre=None):
                """Layer-0 gate A for chains (c0,c0+1): proj+sigmoid+r*h.
                vq: dec -> {'vq': [4 V tiles]}; enc -> {'xca': [...], 'xcb': [...]}.
                Returns (rh_pair, ru_u)."""
                ru_u = {}
                rhpr = rhp.tile([P, N], F16, tag='rh')
                for c in (c0, c0 + 1):
                    if pa_pre is not None:
                        pa = pa_pre[c - c0]
                        proj_cont(pa, slice(0, P),
                                  [(cst[L + '_waq'][:, m, c, :], vq['vq'][m])
                                   for m in range(4)])
                    else:
                        pa = prj.tile([P, N], F32, tag='prj')
                        mms = [(cst[L + '_wa_id'][:], zT0[c])]
                        if L == 'dec0':
                            mms.append((cst['dec0_wa0y'][:], ytmp[c]))
                        else:
                            mms.append((ident[:], vq['xca'][c]))
                        if not skip_h:
                            for m in range(4):
                                mms.append((cst[L + '_wa_hs'][:, m, :],
                                            vl1[(c, m)]))
                        if L == 'dec0':
                            for m in range(4):
                                mms.append((cst[L + '_waq'][:, m, c, :],
                                            vq['vq'][m]))
                        proj(pa, slice(0, P), mms)
                    rr, ru_u[c] = sigmoid_ru(L, pa)
                    off = (c - c0) * HID
                    nc.vector.tensor_mul(rhpr[off:off + HID, :], rr[:],
                                         zT0[c][0:HID, :])
                return rhpr, ru_u

            def gA_l1(L, c0):
                """Layer-1 gate A: diffuse zn1, proj+sigmoid+r*h."""
                wa = cst[L + '_wa']
                for c in (c0, c0 + 1):
                    for m in range(4):
                        vl1[(c, m)] = diffuse(
                            lambda kt, c=c: zn1[c][:, kt, :], m, vlp, 'vl')
                ru_u = {}
                rhpr = rhp.tile([P, N], F16, tag='rh')
                for c in (c0, c0 + 1):
                    pa = prj.tile([P, N], F32, tag='prj')
                    mms = [(wa[0:HID, 0, :], zT1[c][0:HID, :]),
                           (cst[L + '_wa0x'][:], zT0[c][0:HID, :])]
                    for m in range(4):
                        mms.append((wa[:, m + 1, :], vl1[(c, m)]))
                    proj(pa, slice(0, P), mms)
                    rr, ru_u[c] = sigmoid_ru(L, pa)
                    off = (c - c0) * HID
                    nc.vector.tensor_mul(rhpr[off:off + HID, :], rr[:],
                                         zT1[c][0:HID, :])
                return rhpr, ru_u

            def gB_diff(pair, rhpr):
                """Transpose rh pair-pack to node-major, diffuse it."""
                ptr = pdp.tile([P, KT, P], F16, tag='pd')
                for kt in range(KT):
                    nc.tensor.transpose(ptr[:, kt, :],
                                        rhpr[:, kt * P:(kt + 1) * P], ident[:])
                nc.vector.tensor_copy(zr[pair][:], ptr[:])
                return [diffuse(lambda kt, pr=pair: zr[pr][:, kt, :],
                                m, vpp, 'vp') for m in range(4)]

            def gB_proj(L, c0, rhpr, vq, vps):
                """Pair-packed gate-B projection + per-chain tanh."""
                pair = c0 // 2
                pb = prj.tile([P, N], F32, tag='prj')
                mms = [(cst[L + '_wc0h'][:], rhpr)]
                for c in (c0, c0 + 1):
                    hh = c - c0
                    if L == 'enc0':
                        mms.append((cst['enc0_wc0x'][:, hh, :], zT0[c][:]))
                    elif L == 'dec0':
                        mms.append((cst['dec0_wc0y'][:, hh, :], ytmp[c]))
                    else:
                        mms.append((cst[L + '_wc0x'][:, hh, :], zT0[c][0:HID, :]))
                if vq is not None and 'xcb' in vq:
                    mms.append((ident[:], vq['xcb'][pair]))
                for m in range(4):
                    if vq is not None:
                        if 'vq' in vq:
                            mms.append((cst[L + '_wcq'][:, m, pair, :],
                                        vq['vq'][m]))
                    else:
                        for c in (c0, c0 + 1):
                            hh = c - c0
                            mms.append((cst[L + '_wcx'][:, m, hh, :], vl1[(c, m)]))
                    mms.append((cst[L + '_wcr'][:, m, :], vps[m]))
                proj(pb, slice(0, P), mms)
                ct = {}
                for c in (c0, c0 + 1):
                    hh = c - c0
                    ct_c = ctp.tile([HID, N], F16, tag='ct')
                    nc.scalar.activation(ct_c[:], pb[hh * HID:(hh + 1) * HID, :],
                                         AF.Tanh, bias=cst[L + '_bc'][:])
                    ct[c] = ct_c
                return ct

            def step(L0, L1, vq, skip_h, y_t=None, pf0=None, pf1=None,
                     pa_pre=None):
                A0 = [gA_l0(L0, 2 * pr, vq, skip_h,
                            pa_pre if pr == 0 else None) for pr in range(2)]
                C0 = []
                for pr in range(2):
                    vps = gB_diff(pr, A0[pr][0])
                    C0.append(gB_proj(L0, 2 * pr, A0[pr][0], vq, vps))
                if pf0 is not None:
                    pf0()
                A1 = []
                for pr in range(2):
                    for c in (2 * pr, 2 * pr + 1):
                        gru_and_transpose(zT0[c], C0[pr][c], A0[pr][1][c],
                                          zn1[c], HID)
                    A1.append(gA_l1(L1, 2 * pr))
                C1 = []
                for pr in range(2):
                    vps = gB_diff(pr, A1[pr][0])
                    C1.append(gB_proj(L1, 2 * pr, A1[pr][0], None, vps))
                if pf1 is not None:
                    pf1()
                for pr in range(2):
                    for c in (2 * pr, 2 * pr + 1):
                        gru_and_transpose(zT1[c], C1[pr][c], A1[pr][1][c],
                                          zn1[c], 0)
                        if y_t is not None:
                            y_chain(y_t, c)

            def y_chain(t, c):
                py = prj.tile([1, N], F32, tag='prj')
                for ms in range(2):
                    sl = slice(ms * 512, (ms + 1) * 512)
                    nc.tensor.matmul(py[:, sl], cst['fcnw1'][:],
                                     zT1[c][0:HID, sl],
                                     start=True, stop=True)
                nc.scalar.activation(ytmp[c][:], py[:], AF.Identity,
                                     bias=cst['fcnb1'][:])
                nc.sync.dma_start(d_out[t, c:c + 1, :], ytmp[c][:])
                if t < HOR - 1:
                    pqc = pdp.tile([P, KT, 1], F32, tag='pd')
                    for kt in range(KT):
                        nc.tensor.matmul(pqc[:, kt, :],
                                         zT1[c][0:HID, kt * P:(kt + 1) * P],
                                         cst['fcnw1'][:], start=True, stop=True)
                    nc.scalar.activation(yq[:, :, c:c + 1], pqc[:],
                                         AF.Identity, bias=cst['fcnb128'][:])

            # ================= encoder =================
            def load_xc(t, which):
                out = []
                if which == 'a':
                    for c in range(CHAINS):
                        v = xcp.tile([P, N], F16, tag='xc')
                        nc.sync.dma_start(v[:], d['xca'][t, c])
                        out.append(v)
                else:
                    for pr in range(2):
                        v = xcp.tile([P, N], F16, tag='xc')
                        nc.sync.dma_start(v[:], d['xcb'][t, pr])
                        out.append(v)
                return out

            vq = {'xca': load_xc(0, 'a'), 'xcb': load_xc(0, 'b')}
            for c in range(CHAINS):
                nc.sync.dma_start(zT0[c][HID:HID + 2, :], d['xT'][c, 0])
            for t in range(T):
                nxt = {}

                def pre_fin_l0(tt=t, nxt=nxt):
                    if tt + 1 < T:
                        for c in range(CHAINS):
                            nc.sync.dma_start(zT0[c][HID:HID + 2, :],
                                              d['xT'][c, tt + 1])
                        nxt['xca'] = load_xc(tt + 1, 'a')

                def pre_fin_l1(tt=t, nxt=nxt):
                    if tt + 1 < T:
                        nxt['xcb'] = load_xc(tt + 1, 'b')

                step('enc0', 'enc1', vq, skip_h=(t == 0),
                     pf0=pre_fin_l0, pf1=pre_fin_l1)
                if t + 1 < T:
                    vq = {'xca': nxt['xca'], 'xcb': nxt['xcb']}

            # ================= decoder =================
            for c in range(CHAINS):
                nc.sync.dma_start(zT0[c][HID:HID + 2, :], d_zeros[0:2, :])
                nc.sync.dma_start(ytmp[c][:], d_zeros[0:1, :])
            nc.sync.dma_start(
                yq[:], d_zeros[:, 0:KT * CHAINS].rearrange(
                    'p (kt f) -> p kt f', kt=KT))
            for t in range(HOR):
                pa_pre = []
                for c in range(2):
                    pa = prj.tile([P, N], F32, tag='prj')
                    mms = [(cst['dec0_wa_id'][:], zT0[c]),
                           (cst['dec0_wa0y'][:], ytmp[c])]
                    for m in range(4):
                        mms.append((cst['dec0_wa_hs'][:, m, :], vl1[(c, m)]))
                    proj(pa, slice(0, P), mms)
                    pa_pre.append(pa)
                vq = {'vq': [diffuse(lambda kt: yq[:, kt, :], m, xcp, 'xc',
                             nfeat=CHAINS) for m in range(4)]}
                step('dec0', 'dec1', vq, skip_h=False, y_t=t, pa_pre=pa_pre)

    print(f'[build] emission+schedule: {_time.time() - _t0:.1f}s', flush=True)
    _t1 = _time.time()
    nc.compile()
    print(f'[build] bacc compile: {_time.time() - _t1:.1f}s', flush=True)
    return nc


def _prep_host(inputs):
    """Host-side preprocessing -> per-core in_maps (fp16)."""
    f32 = np.float32
    f16 = np.float16
    adj = np.asarray(inputs['adj'], f32)
    source = np.asarray(inputs['source'], f32)       # [B, T, N, 2]

    def rw(a):
        return (a / np.maximum(a.sum(1, keepdims=True), np.float32(1e-8))).astype(f32)

    s0 = rw(adj)
    s1 = rw(adj.T)
    mats = [s0, s0 @ s0, s1, s1 @ s1]

    def tile_nm(a):
        return np.ascontiguousarray(
            a.reshape(KT, P, N).transpose(1, 0, 2)).astype(f16)

    common = {f's_t{m}': tile_nm(mats[m]) for m in range(4)}
    mats16 = [mm_.astype(f16).astype(f32) for mm_ in mats]
    common['ident'] = np.eye(P, dtype=f16)

    def basis(Wname, I):
        F = I + HID
        W = np.asarray(inputs[Wname], f32)
        b = [W[i * F:(i + 1) * F] for i in range(5)]
        return [b[0] - b[2] - b[4], b[1], 2 * b[2], b[3], 2 * b[4]]

    for L, I in (('enc0', 2), ('dec0', 1)):
        F = I + HID
        eA = basis(L + '_Wru', I)
        eC = basis(L + '_Wc', I)
        wa_id = np.zeros((P, P), f32)
        wa_id[0:HID] = eA[0][I:F]
        if L == 'enc0':
            wa_id[HID:HID + I] = eA[0][0:I]
        common[L + '_wa_id'] = wa_id.astype(f16)
        wa_hs = np.zeros((P, 4, P), f32)
        for m in range(4):
            wa_hs[HID:P, m] = eA[m + 1][I:F]
        common[L + '_wa_hs'] = wa_hs.astype(f16)
        if L == 'dec0':
            waq = np.zeros((4 * I, 4, CHAINS, P), f32)
            wcq = np.zeros((4 * I, 4, 2, P), f32)
            for m in range(4):
                for c in range(CHAINS):
                    waq[c * I:(c + 1) * I, m, c] = eA[m + 1][0:I]
                    pr, hh = c // 2, c % 2
                    wcq[c * I:(c + 1) * I, m, pr,
                        hh * HID:(hh + 1) * HID] = eC[m + 1][0:I]
            common[L + '_waq'] = waq.astype(f16)
            common[L + '_wcq'] = wcq.astype(f16)
        else:
            enc0_eA = [e.astype(f16).astype(f32) for e in eA]
            enc0_eC = [e.astype(f16).astype(f32) for e in eC]
            _cache['enc0_bases'] = (enc0_eA, enc0_eC, I)
        wc0h = np.zeros((P, P), f32)
        wcr = np.zeros((P, 4, P), f32)
        for hh in range(2):
            cs = slice(hh * HID, (hh + 1) * HID)
            wc0h[cs, cs] = eC[0][I:F]
            for m in range(4):
                wcr[cs, m, cs] = eC[m + 1][I:F]
        common[L + '_wc0h'] = wc0h.astype(f16)
        common[L + '_wcr'] = wcr.astype(f16)
        bru = np.asarray(inputs[L + '_bru'], f32)
        common[L + '_bru'] = np.stack([bru[0:HID], bru[HID:P]], axis=1).copy()
        common[L + '_bc'] = np.asarray(inputs[L + '_bc'], f32).reshape(HID, 1).copy()
        if L == 'enc0':
            wc0x = np.zeros((P, 2, P), f32)
            for hh in range(2):
                wc0x[HID:HID + I, hh, hh * HID:(hh + 1) * HID] = eC[0][0:I]
            common['enc0_wc0x'] = wc0x.astype(f16)
        else:
            common['dec0_wa0y'] = np.ascontiguousarray(eA[0][0:1]).astype(f16)
            wc0y = np.zeros((1, 2, P), f32)
            for hh in range(2):
                wc0y[0, hh, hh * HID:(hh + 1) * HID] = eC[0][0:1]
            common['dec0_wc0y'] = wc0y.astype(f16)

    for L, I in L1LAYERS:
        F = I + HID
        eA = basis(L + '_Wru', I)
        eC = basis(L + '_Wc', I)
        wa = np.zeros((P, 5, P), f32)
        for i in range(5):
            wa[0:HID, i] = eA[i][I:F]
            if i > 0:
                wa[HID:P, i] = eA[i][0:I]
        common[L + '_wa'] = wa.astype(f16)
        common[L + '_wa0x'] = np.ascontiguousarray(eA[0][0:I]).astype(f16)
        wc0h = np.zeros((P, P), f32)
        wcr = np.zeros((P, 4, P), f32)
        for hh in range(2):
            cs = slice(hh * HID, (hh + 1) * HID)
            wc0h[cs, cs] = eC[0][I:F]
            for m in range(4):
                wcr[cs, m, cs] = eC[m + 1][I:F]
        common[L + '_wc0h'] = wc0h.astype(f16)
        common[L + '_wcr'] = wcr.astype(f16)
        wc0x = np.zeros((HID, 2, P), f32)
        wcx = np.zeros((P, 4, 2, P), f32)
        for hh in range(2):
            cs = slice(hh * HID, (hh + 1) * HID)
            wc0x[:, hh, cs] = eC[0][0:I]
            for m in range(4):
                wcx[HID:P, m, hh, cs] = eC[m + 1][0:I]
        common[L + '_wc0x'] = wc0x.astype(f16)
        common[L + '_wcx'] = wcx.astype(f16)
        bru = np.asarray(inputs[L + '_bru'], f32)
        common[L + '_bru'] = np.stack([bru[0:HID], bru[HID:P]], axis=1).copy()
        common[L + '_bc'] = np.asarray(inputs[L + '_bc'], f32).reshape(HID, 1).copy()

    common['zeros'] = np.zeros((P, N), f16)
    fw = np.asarray(inputs['fcn_W'], f32).reshape(HID, 1)
    common['fcnw1'] = fw.astype(f16).copy()
    fb = np.asarray(inputs['fcn_b'], f32).reshape(-1)[0]
    common['fcnb1'] = np.full((1, 1), fb, f32)
    common['fcnb128'] = np.full((P, 1), fb, f32)

    T = _cache['T']
    in_maps = []
    for core in range(N_CORES):
        m = dict(common)
        eA0, eC0, I0 = _cache['enc0_bases']
        xT = np.zeros((CHAINS, T, 2, N), f16)
        xca = np.zeros((T, CHAINS, P, N), f16)
        xcb = np.zeros((T, 2, P, N), f16)
        for t in range(T):
            for c in range(CHAINS):
                b = core * CHAINS + c
                xc16 = source[b, t].astype(f16).astype(f32)   # [N, 2]
                xT[c, t] = xc16.T.astype(f16)
                suma = np.zeros((P, N), f32)
                sumb = np.zeros((HID, N), f32)
                for mm_ in range(4):
                    Vm = xc16.T @ mats16[mm_]                 # [2, N]
                    Vm = Vm.astype(f16).astype(f32)
                    suma += eA0[mm_ + 1][0:I0].T @ Vm
                    sumb += eC0[mm_ + 1][0:I0].T @ Vm
                xca[t, c] = suma.astype(f16)
                pr, hh = c // 2, c % 2
                xcb[t, pr, hh * HID:(hh + 1) * HID] = sumb.astype(f16)
        m['xT'] = xT
        m['xca'] = xca
        m['xcb'] = xcb
        in_maps.append(m)
    return in_maps


def kernel(**inputs):
    from concourse import bass_utils

    T = int(os.environ.get('DCRNN_T', T_FULL))
    HOR = int(os.environ.get('DCRNN_HOR', HOR_FULL))
    key = (T, HOR)
    if _cache.get('key') != key:
        _cache['nc'] = _build(T, HOR)
        _cache['key'] = key
    _cache['T'] = T
    _cache['HOR'] = HOR

    in_maps = _prep_host(inputs)
    res = bass_utils.run_bass_kernel_spmd(
        _cache['nc'], in_maps, core_ids=list(range(N_CORES)),
        trace=bool(int(os.environ.get('DCRNN_TRACE', '0'))))
    _cache['last_res'] = res

    HORr = _cache['HOR']
    out = np.zeros((HORr, B, N), np.float32)
    for core in range(N_CORES):
        r = res.results[core]['out']         # [HOR, CHAINS, N] fp16
        for c in range(CHAINS):
            out[:, core * CHAINS + c, :] = r[:, c, :].astype(np.float32)
    return out
